# revision 21
# baseline (speedup 1.0000x reference)
"""Multi-head causal self-attention (B=2, S=2048, H=2048, NH=16) on 8 TRN2
NeuronCores.

Sharding: data-parallel over batch (2 groups of 4 cores) x tensor-parallel
over heads (4 heads per core; q/k/v projections column-split, output
projection row-split). Each core computes a partial [S, H] output-projection
product; the host sums the 4 partials per batch and adds the output bias.

Per-core device kernel (all matmul inputs bf16, fp32 PSUM accumulation),
organized as one software-pipelined stream so the 2.4 GHz PE never waits
on the 1.2 GHz ACT engine or on DVE drains:
  - attention chunks (head, 512-wide q-chunk) in [k, q] orientation, one
    k-tile (128 keys) per single-bank PSUM tile from a 5-deep ring; the
    four diagonal k-tiles trim scores/exp/pv to the causal q-range
    (q_loc >= 128*t_loc), so only a shared [128,128] triangle mask ever
    multiplies; scores of tile k are emitted before the pv matmul of
    tile k-2
  - softmax denominator: bf16 pair-sums accumulated into an f32r esum on
    DVE (range-split on the diagonal pairs), then one f32r matmul per
    chunk whose all-ones [128,128] stationary operand reduces over
    partitions AND broadcasts the result; den matmul and the
    reciprocal+multiply normalize are deferred into the NEXT chunk's
    stream
  - j=0 phase-1 runs contraction-tile-major (4 PSUM accumulators per
    pass) so the cold-start PE idle is bounded by the DMA lead of one
    tile; later Q/K/V projection chains (1/sqrt(hd) folded into Wq on
    host) for round j+1 and output-projection row-tiles for round j-1
    ride as "filler quanta" popped between attention k-steps
  - the final den/normalize latency is hidden behind head-0..2 partial
    products of the first tail outproj quanta
  - PSUM->SBUF output drains alternate between ACT and DVE

_build_nc(reps=K) repeats the identical computation K times in one NEFF;
test.py measures the per-execution slope between two reps builds, which
cancels the multi-ms axon/PJRT dispatch overhead that would otherwise
swamp the ~285 us device time.
"""

import math
import sys

if "/opt/trn_rl_repo" not in sys.path:
    sys.path.insert(0, "/opt/trn_rl_repo")

import numpy as np
import ml_dtypes

import concourse.bass as bass
import concourse.mybir as mybir
import concourse.tile as tile
from concourse.bass_utils import run_bass_kernel_spmd

B, S, H, NH = 2, 2048, 2048, 16
HD = H // NH            # 128
NCORES = 8
HPC = NH // 4           # 4 heads per core
DSH = HPC * HD          # 512 per-core head-dim shard
P = 128                 # partitions
NT = S // P             # 16 s/k tiles of 128
NJ = S // 512           # 4 q/s chunks of 512
BF16 = mybir.dt.bfloat16
F32 = mybir.dt.float32

_NEG_BIG = -1.0e8  # masked entries in the reference mask are <= -1e9


def _split_excess_waits(nc, max_waits: int = 1) -> int:
    """This container's walrus rejects >1 sync wait per instruction
    ("Too many sync wait commands" in setupSyncWait). Hoist excess waits
    onto preceding same-engine NoOps; waits still execute in engine order
    before the original instruction, so sync semantics are unchanged."""
    n_split = 0
    for f in nc.m.functions:
        for bb in f.blocks:
            insts = bb.instructions
            out = []
            changed = False
            for inst in insts:
                si = inst.sync_info
                if si is not None and len(si.on_wait) > max_waits:
                    waits = list(si.on_wait)
                    excess, keep = waits[:-max_waits], waits[-max_waits:]
                    for i in range(0, len(excess), max_waits):
                        chunk = excess[i : i + max_waits]
                        nop = mybir.InstNoOp(
                            name=f"{inst.name}-waitsplit-{i}", ins=[], outs=[]
                        )
                        nop.engine = inst.engine
                        nop.sync_info = mybir.SyncInfo(on_wait=chunk, on_update=[])
                        nc.register_instruction(nop)
                        out.append(nop)
                    inst.sync_info = mybir.SyncInfo(
                        on_wait=keep, on_update=list(si.on_update)
                    )
                    changed = True
                    n_split += 1
                out.append(inst)
            if changed:
                bb.instructions = out
    return n_split


def _build_nc(reps: int = 1):
    """Build the kernel IR. reps>1 repeats the whole computation (including
    all DMA) back-to-back inside one NEFF — used by the timing harness to
    amortize the per-dispatch overhead of the axon/PJRT path; every rep
    performs identical work to the reps=1 program kernel() executes."""
    nc = bass.Bass()
    ht = nc.dram_tensor("ht", (H, S), BF16, kind="ExternalInput")
    wqt = nc.dram_tensor("wqt", (H, DSH), BF16, kind="ExternalInput")
    wkt = nc.dram_tensor("wkt", (H, DSH), BF16, kind="ExternalInput")
    wvt = nc.dram_tensor("wvt", (H, DSH), BF16, kind="ExternalInput")
    wot = nc.dram_tensor("wot", (DSH, H), BF16, kind="ExternalInput")
    bq2 = nc.dram_tensor("bq2", (P, HPC), F32, kind="ExternalInput")
    bk2 = nc.dram_tensor("bk2", (P, HPC), F32, kind="ExternalInput")
    bvb = nc.dram_tensor("bvb", (P, DSH), F32, kind="ExternalInput")
    msk = nc.dram_tensor("msk", (P, 4, 512), BF16, kind="ExternalInput")
    # partial products are summed on the host; bf16 partials halve the
    # output traffic and cost <0.1% relative error on the final sum
    o = nc.dram_tensor("o", (S, H), BF16, kind="ExternalOutput")

    with tile.TileContext(nc) as tc:
        with (
            tc.tile_pool(name="wpool", bufs=1) as wpool,
            tc.tile_pool(name="cpool", bufs=1) as cpool,
            tc.tile_pool(name="hpool", bufs=2) as hpool,
            tc.tile_pool(name="qkpool", bufs=1) as qkpool,
            tc.tile_pool(name="epool", bufs=6) as epool,
            tc.tile_pool(name="rpool", bufs=2) as rpool,
            tc.tile_pool(name="opool", bufs=4) as opool,
            # one shared pool of single-bank score/acc tiles + ot
            # accumulators + the den broadcast: 5 + 2 + 1 = 8 PSUM banks
            tc.tile_pool(name="ps_mm", bufs=5, space="PSUM") as ps_mm,
            tc.tile_pool(name="ps_out", bufs=2, space="PSUM") as ps_out,
            tc.tile_pool(name="ps_den", bufs=1, space="PSUM") as ps_den,
        ):
            for _rep in range(reps):
                _build_body(
                    nc, wpool, cpool, hpool, qkpool, epool, rpool, opool,
                    ps_mm, ps_out, ps_den,
                    ht, wqt, wkt, wvt, wot, bq2, bk2, bvb, msk, o,
                )

    _split_excess_waits(nc)
    return nc


def _build_body(
    nc, wpool, cpool, hpool, qkpool, epool, rpool, opool,
    ps_mm, ps_out, ps_den,
    ht, wqt, wkt, wvt, wot, bq2, bk2, bvb, msk, o,
):
    if True:
        if True:
            # ---- constants / weights into SBUF ----
            # Load order matters: the first Q matmuls need wq + the first
            # hidden chunk; split the big loads in 4 so they spread across
            # DMA queues and compute starts as early as possible. wo is not
            # needed until phase 3 and is loaded right before it.
            wq_sb = wpool.tile([P, NT, DSH], BF16, tag="wq")
            wk_sb = wpool.tile([P, NT, DSH], BF16, tag="wk")
            wv_sb = wpool.tile([P, NT, DSH], BF16, tag="wv")
            wqt_r = wqt.rearrange("(t p) m -> p t m", p=P)
            wkt_r = wkt.rearrange("(t p) m -> p t m", p=P)
            wvt_r = wvt.rearrange("(t p) m -> p t m", p=P)
            # finest split for the first-needed tiles, alternating the two
            # tensors the first accumulation reads: the j=0 Q pass can begin
            # as soon as hidden tile 0 + wq tile 0 arrive
            h0_sb = hpool.tile([P, NT, 512], BF16, tag="h")
            ht_r0 = ht[:, 0:512].rearrange("(t p) s -> p t s", p=P)
            for t in range(NT):
                nc.sync.dma_start(h0_sb[:, t, :], ht_r0[:, t, :])
                nc.sync.dma_start(wq_sb[:, t, :], wqt_r[:, t, :])
            bq_sb = cpool.tile([P, HPC], F32, tag="bq")
            nc.sync.dma_start(bq_sb[:], bq2[:, :])
            bk_sb = cpool.tile([P, HPC], F32, tag="bk")
            nc.sync.dma_start(bk_sb[:], bk2[:, :])
            bv_sb = cpool.tile([P, DSH], F32, tag="bv")
            nc.sync.dma_start(bv_sb[:], bvb[:, :])
            mask_sb = cpool.tile([P, 4, 512], BF16, tag="msk")
            nc.sync.dma_start(mask_sb[:], msk[:, :, :])
            # all-ones [128,128] f32r matrix: the den matmul's stationary
            # operand, so its partition-reduction lands already broadcast
            # across all 128 output partitions (no separate rb matmul)
            ones_f32 = cpool.tile([P, P], F32, tag="ones32")
            nc.vector.memset(ones_f32[:], 1.0)
            onesmat_sb = cpool.tile([P, P], mybir.dt.float32r, tag="ones")
            nc.vector.tensor_copy(onesmat_sb[:], ones_f32[:])

            qt_sb = qkpool.tile([P, HPC, S], BF16, tag="qt")   # per-head Q^T [d, s]
            kt_sb = qkpool.tile([P, HPC, S], BF16, tag="kt")   # per-head K^T [d, s]
            v_sb = qkpool.tile([P, NT, DSH], BF16, tag="v")   # V [s-tile, d]
            ao_sb = qkpool.tile([P, HPC, S], BF16, tag="ao")   # attn-out^T [d, q] per head

            wo_sb = wpool.tile([P, HPC, H], BF16, tag="wo")
            wot_r = wot.rearrange("(t p) h -> p t h", p=P)

            h_tiles = {}

            def _phase1_dma(j):
                # kick off the hidden-chunk (and, for j=0, weight) loads
                sj = slice(512 * j, 512 * (j + 1))
                if j == 0:
                    h_tiles[0] = h0_sb
                    # K/V weights arrive while the j=0 Q pass computes; wo
                    # is issued in _phase1_dma(1) AFTER the h1 chunk, since
                    # h1 feeds round-0 fillers (~37us) while wo is first
                    # read by outproj(0) during round 1 (~70us)
                    for q4 in range(4):
                        t4 = slice(4 * q4, 4 * (q4 + 1))
                        nc.sync.dma_start(wk_sb[:, t4, :], wkt_r[:, t4, :])
                    for q4 in range(4):
                        t4 = slice(4 * q4, 4 * (q4 + 1))
                        nc.sync.dma_start(wv_sb[:, t4, :], wvt_r[:, t4, :])
                else:
                    h_sb = hpool.tile([P, NT, 512], BF16, tag="h")
                    ht_r = ht[:, sj].rearrange("(t p) s -> p t s", p=P)
                    for q4 in range(4):
                        t4 = slice(4 * q4, 4 * (q4 + 1))
                        nc.sync.dma_start(h_sb[:, t4, :], ht_r[:, t4, :])
                    h_tiles[j] = h_sb
                    if j == 1:
                        for q4 in range(4):
                            nc.sync.dma_start(wo_sb[:, q4, :], wot_r[:, q4, :])

            # ---- filler quanta ----
            # Phase-1 projection chains and output-projection groups are
            # wrapped in small closures ("quanta", ~0.9-3.4us of PE work
            # each) and popped from a FIFO after every attention pair-step.
            # These PE-only stretches between score/PV steps let the 1.2 GHz
            # ACT engine (which can never outrun the 2.4 GHz PE within a
            # contiguous run of attention steps) catch up on its exp
            # backlog, and widen every PSUM-buffer / DVE-drain recycle
            # window. Accumulators are single-bank [P,512] tiles from the
            # shared ps_mm ring.
            filler_q = []

            def _fill_half():
                return ps_mm.tile([P, 512], F32, tag="mm", name="fillacc")

            def _pop_filler(n=1):
                for _ in range(n):
                    if not filler_q:
                        return
                    filler_q.pop(0)()

            def _drain_fillers():
                while filler_q:
                    filler_q.pop(0)()

            def _q_quantum(j, hd, h_sb):
                def f():
                    sj = slice(512 * j, 512 * (j + 1))
                    md = slice(HD * hd, HD * (hd + 1))
                    acc = _fill_half()
                    for t in range(NT):
                        nc.tensor.matmul(
                            acc, wq_sb[:, t, md], h_sb[:, t, :],
                            start=(t == 0), stop=(t == NT - 1),
                            skip_group_check=True,
                        )
                    nc.vector.tensor_scalar_add(
                        qt_sb[:, hd, sj], acc, bq_sb[:, hd : hd + 1]
                    )
                return f

            def _k_quantum(j, hd, h_sb):
                def f():
                    sj = slice(512 * j, 512 * (j + 1))
                    md = slice(HD * hd, HD * (hd + 1))
                    acc = _fill_half()
                    for t in range(NT):
                        nc.tensor.matmul(
                            acc, wk_sb[:, t, md], h_sb[:, t, :],
                            start=(t == 0), stop=(t == NT - 1),
                            skip_group_check=True,
                        )
                    nc.vector.tensor_scalar_add(
                        kt_sb[:, hd, sj], acc, bk_sb[:, hd : hd + 1]
                    )
                return f

            def _v_quantum(j, st, h_sb):
                def f():
                    ms = slice(P * st, P * (st + 1))
                    acc = _fill_half()
                    for t in range(NT):
                        nc.tensor.matmul(
                            acc, h_sb[:, t, ms], wv_sb[:, t, :],
                            start=(t == 0), stop=(t == NT - 1),
                            skip_group_check=True,
                        )
                    nc.vector.tensor_add(v_sb[:, 4 * j + st, :], acc, bv_sb[:])
                return f

            _oc_flip = [0]
            _tail_mode = [False]

            def _oc_drain(acc, rs, hc):
                oc = opool.tile([P, 512], BF16, tag="oc")
                # PSUM->SBUF drains alternate ACT/DVE so neither queue
                # grows enough to delay release-critical ops (exp on
                # ACT; esum / normalize on DVE). In the post-attention
                # tail both exp and normalize are done, and DVE still
                # holds hoisted waits — route every drain to the idle ACT
                # so accumulator banks recycle as fast as possible.
                _oc_flip[0] ^= 1
                if _tail_mode[0] or _oc_flip[0]:
                    nc.scalar.activation(
                        oc[:], acc, mybir.ActivationFunctionType.Copy
                    )
                else:
                    nc.vector.tensor_copy(oc[:], acc)
                nc.sync.dma_start(o[rs, hc], oc[:])

            def _o_quantum(si, c):
                def f():
                    rs = slice(P * si, P * (si + 1))
                    hc = slice(512 * c, 512 * (c + 1))
                    acc = _fill_half()
                    for dt in range(HPC):
                        nc.tensor.matmul(
                            acc, ao_sb[:, dt, rs], wo_sb[:, dt, hc],
                            start=(dt == 0), stop=(dt == HPC - 1),
                            skip_group_check=True,
                        )
                    _oc_drain(acc, rs, hc)
                return f

            def _queue_phase1(j):
                # kind-major order: at j=0 this matches the DMA arrival
                # order (wq -> wk -> wv), so the PE never waits on a weight
                # tensor whose load was issued after one it already consumed
                h_sb = h_tiles.pop(j)
                for hd in range(HPC):
                    filler_q.append(_q_quantum(j, hd, h_sb))
                for hd in range(HPC):
                    filler_q.append(_k_quantum(j, hd, h_sb))
                for hd in range(HPC):
                    filler_q.append(_v_quantum(j, hd, h_sb))

            def _queue_outproj(j):
                for si in range(4 * j, 4 * (j + 1)):
                    for c in range(NJ):
                        filler_q.append(_o_quantum(si, c))

            # pending[0]: previous chunk awaiting its den matmul + normalize;
            # both are deferred into the NEXT chunk's PE stream so the PE
            # never stalls on the ACT-exp / DVE-esum latency behind them
            pending = [None]

            def _flush_den():
                # den matmul for the previous chunk: ONE f32r matmul whose
                # all-ones [128,128] stationary operand both reduces the
                # DVE-accumulated esum over partitions AND broadcasts the
                # result to every output partition (kmax x cheaper on PE
                # than [1,512] ones-matmuls per k-tile, and no rb matmul)
                ot_ps, esum_r, n_hd, n_sj, den_ref = pending[0]
                den_ps = ps_den.tile([P, 512], F32, tag="den")
                nc.tensor.matmul(
                    den_ps[:], onesmat_sb[:], esum_r[:], start=True, stop=True
                )
                den_ref.append(den_ps)

            def _normalize():
                # divide the accumulated outT by the softmax denominator:
                # reciprocal of the broadcast den on DVE, then a multiply
                # into the bf16 attn-out tile (reciprocal_approx_fast would
                # shave ~500ns but its custom DVE ISA op fails this
                # container's walrus codegen: "ISA wrong length")
                ot_ps, _, n_hd, n_sj, den_ref = pending[0]
                rb = rpool.tile([P, 512], F32, tag="rb")
                nc.vector.reciprocal(rb[:], den_ref[0][:])
                nc.vector.tensor_mul(ao_sb[:, n_hd, n_sj], ot_ps[:], rb[:])
                pending[0] = None

            def _attn_chunk(hd, j):
                # causal attention for (head hd, q-chunk j), [k, q]
                # orientation, one k-tile (128 keys) per PSUM bank. The four
                # diagonal k-tiles (the last four) restrict the moving q
                # range to q_loc >= 128*t_loc: scores, exp and pv all skip
                # the strictly-upper-triangle 512-blocks, and only the
                # 128-wide diagonal block needs a (shared) triangle mask.
                sj0 = 512 * j
                kmax = 4 * (j + 1)
                md = slice(HD * hd, HD * (hd + 1))
                trims = [max(0, 128 * (k - (kmax - 4))) for k in range(kmax)]
                flush_k = min(5, kmax - 1)
                ot_ps = ps_out.tile([P, 512], F32, tag="ot")
                # accumulated directly in f32r (the den matmul's moving
                # operand dtype): every DVE writer rounds to f32r, so no
                # converting copy is needed before the den matmul
                esum = rpool.tile([P, 512], mybir.dt.float32r, tag="esum")
                # PE stream is in-order: emit the scores of tile k BEFORE the
                # pv matmul of tile k-2 so the PE streams scores while ACT
                # computes exp, and slot the previous chunk's den matmul /
                # normalize into fixed early positions
                etiles = [None] * kmax
                for k in range(kmax + 2):
                    if k < kmax:
                        a = trims[k]
                        kd = slice(P * k, P * (k + 1))
                        st = ps_mm.tile([P, 512], F32, tag="mm")
                        nc.tensor.matmul(
                            st[:, a:], kt_sb[:, hd, kd],
                            qt_sb[:, hd, sj0 + a : sj0 + 512],
                            start=True, stop=True,
                            skip_group_check=True,
                        )
                        e = epool.tile([P, 512], BF16, tag="e")
                        nc.scalar.activation(
                            e[:, a:], st[:, a:],
                            mybir.ActivationFunctionType.Exp,
                        )
                        if k >= kmax - 4:
                            # triangle mask on the 128-wide diagonal block
                            nc.vector.tensor_mul(
                                e[:, a : a + 128], e[:, a : a + 128],
                                mask_sb[:, 0, 0:128],
                            )
                        etiles[k] = e
                        if k % 2 == 1:
                            # denominator: bf16 pair-sum over the region both
                            # tiles cover, direct add where only the earlier
                            # tile is valid, then f32 accumulate
                            ep, ac = etiles[k - 1], a
                            pb = rpool.tile([P, 512], BF16, tag="pb")
                            if ac == 0:
                                nc.vector.tensor_add(pb[:], ep[:], e[:])
                                if k == 1:
                                    nc.vector.tensor_copy(esum[:], pb[:])
                                else:
                                    nc.vector.tensor_add(esum[:], esum[:], pb[:])
                            else:
                                ap = ac - 128
                                nc.vector.tensor_add(
                                    pb[:, ac:], ep[:, ac:], e[:, ac:]
                                )
                                if k == 1:
                                    nc.vector.tensor_copy(
                                        esum[:, ap:ac], ep[:, ap:ac]
                                    )
                                    nc.vector.tensor_copy(
                                        esum[:, ac:], pb[:, ac:]
                                    )
                                else:
                                    nc.vector.tensor_add(
                                        esum[:, ap:ac], esum[:, ap:ac],
                                        ep[:, ap:ac],
                                    )
                                    nc.vector.tensor_add(
                                        esum[:, ac:], esum[:, ac:], pb[:, ac:]
                                    )
                        if k == flush_k and pending[0] is not None:
                            _flush_den()
                    if k >= 2:
                        kk = k - 2
                        a = trims[kk]
                        nc.tensor.matmul(
                            ot_ps[:, a:], v_sb[:, kk, md], etiles[kk][:, a:],
                            start=(kk == 0), stop=(kk == kmax - 1),
                            skip_group_check=True,
                        )
                        if kk == flush_k and pending[0] is not None:
                            _normalize()
                            if on_norm is not None:
                                on_norm()
                    if k % 2 == 1:
                        _pop_filler()
                pending[0] = (ot_ps, esum, hd, slice(sj0, sj0 + 512), [])

            def _phase1_j0_tilemajor():
                # j=0 phase-1 in contraction-tile-major order: each arriving
                # (h0[t], w[t]) DMA pair unlocks HPC matmuls immediately, so
                # the cold-start PE idle is bounded by the DMA lead of a
                # single tile instead of a whole tensor. Four accumulators
                # (one per head / s-tile) live in the 5-deep ps_mm ring.
                h_sb = h_tiles.pop(0)
                for kind in range(3):
                    w_sb = (wq_sb, wk_sb, wv_sb)[kind]
                    accs = [
                        ps_mm.tile([P, 512], F32, tag="mm", name="p1acc")
                        for _ in range(HPC)
                    ]
                    for t in range(NT):
                        for i in range(HPC):
                            if kind == 2:
                                # V: stationary = hidden s-tile, moving = wv
                                nc.tensor.matmul(
                                    accs[i], h_sb[:, t, P * i : P * (i + 1)],
                                    w_sb[:, t, :],
                                    start=(t == 0), stop=(t == NT - 1),
                                    skip_group_check=True,
                                )
                            else:
                                nc.tensor.matmul(
                                    accs[i], w_sb[:, t, HD * i : HD * (i + 1)],
                                    h_sb[:, t, :],
                                    start=(t == 0), stop=(t == NT - 1),
                                    skip_group_check=True,
                                )
                    for i in range(HPC):
                        if kind == 0:
                            nc.vector.tensor_scalar_add(
                                qt_sb[:, i, 0:512], accs[i], bq_sb[:, i : i + 1]
                            )
                        elif kind == 1:
                            nc.vector.tensor_scalar_add(
                                kt_sb[:, i, 0:512], accs[i], bk_sb[:, i : i + 1]
                            )
                        else:
                            nc.vector.tensor_add(v_sb[:, i, :], accs[i], bv_sb[:])

            # Software pipeline: phase-1 of round j+1 and the output
            # projection of round j-1 ride as filler quanta inside round j's
            # attention chunks (K/V tiles of round j only reach s <= 512(j+1)
            # by causality, and outproj j-1 unblocks once chunk(0, j) flushes
            # the last pending normalize of round j-1).
            _phase1_dma(0)
            if NJ > 1:
                _phase1_dma(1)
            _phase1_j0_tilemajor()
            for j in range(NJ):
                # h for round j+2 streams in while round j computes; the
                # phase-1 quanta of round j+1 queued here read the h chunk
                # that already landed during round j-1
                if j + 2 < NJ:
                    _phase1_dma(j + 2)
                if j + 1 < NJ:
                    _queue_phase1(j + 1)
                for hd in range(HPC):
                    on_norm = None
                    if hd == 0 and j > 0:
                        jj = j - 1
                        on_norm = lambda jj=jj: _queue_outproj(jj)
                    _attn_chunk(hd, j)
                # round j+1's chunks need all of phase-1(j+1): drain whatever
                # the pair-step slots didn't absorb (outproj quanta may spill)
                if j + 1 < NJ:
                    _drain_fillers()
            # Tail: the final den matmul waits ~1.5us on the DVE esum chain
            # of chunk (3, NJ-1). Emit the head-0..2 partial products of the
            # first few outproj quanta ahead of it in PE program order (they
            # only read already-normalized heads), then finish them (head-3
            # matmul + drain) after the last normalize.
            def _o_quantum_partial(si, c):
                rs = slice(P * si, P * (si + 1))
                hc = slice(512 * c, 512 * (c + 1))
                acc = _fill_half()
                for dt in range(HPC - 1):
                    nc.tensor.matmul(
                        acc, ao_sb[:, dt, rs], wo_sb[:, dt, hc],
                        start=(dt == 0), stop=False,
                        skip_group_check=True,
                    )

                def finish():
                    nc.tensor.matmul(
                        acc, ao_sb[:, HPC - 1, rs], wo_sb[:, HPC - 1, hc],
                        start=False, stop=True,
                        skip_group_check=True,
                    )
                    _oc_drain(acc, rs, hc)

                return finish

            # 5 partials = the ps_mm ring size; a 6th open accumulator would
            # deadlock against its own finisher's drain in the PE FIFO.
            _tail_mode[0] = True
            sis = 4 * (NJ - 1)
            finishers = [_o_quantum_partial(sis, c) for c in range(3)]
            _flush_den()
            _normalize()
            finishers.append(_o_quantum_partial(sis, 3))
            finishers.append(_o_quantum_partial(sis + 1, 0))
            for f in finishers:
                f()
            for c in range(1, NJ):
                filler_q.append(_o_quantum(sis + 1, c))
            for si in range(sis + 2, sis + 4):
                for c in range(NJ):
                    filler_q.append(_o_quantum(si, c))
            _drain_fillers()

    _split_excess_waits(nc)
    return nc


_NC_CACHE = None


def _get_nc():
    global _NC_CACHE
    if _NC_CACHE is None:
        _NC_CACHE = _build_nc()
    return _NC_CACHE


def _is_causal_mask(mask: np.ndarray) -> bool:
    if mask.shape != (1, 1, S, S):
        return False
    m = mask[0, 0]
    tri = np.tril(np.ones((S, S), dtype=bool))
    return bool(np.all(m[tri] == 0.0) and np.all(m[~tri] <= _NEG_BIG))


def _reference_numpy(hidden_states, attention_mask, Wq, bq, Wk, bk, Wv, bv, Wo, bo):
    hs = hidden_states.astype(np.float64)
    out = np.empty((B, S, H), np.float64)
    for b in range(B):
        q = hs[b] @ Wq.T.astype(np.float64) + bq
        k = hs[b] @ Wk.T.astype(np.float64) + bk
        v = hs[b] @ Wv.T.astype(np.float64) + bv
        q = q.reshape(S, NH, HD).transpose(1, 0, 2)
        k = k.reshape(S, NH, HD).transpose(1, 0, 2)
        v = v.reshape(S, NH, HD).transpose(1, 0, 2)
        attn = np.einsum("nqd,nkd->nqk", q, k) / math.sqrt(HD)
        attn = attn + attention_mask[0].astype(np.float64)
        attn = attn - attn.max(axis=-1, keepdims=True)
        attn = np.exp(attn)
        attn = attn / attn.sum(axis=-1, keepdims=True)
        o = np.einsum("nqk,nkd->nqd", attn, v)
        o = o.transpose(1, 0, 2).reshape(S, H)
        out[b] = o @ Wo.T.astype(np.float64) + bo
    return out.astype(np.float32)


def _prepare_in_maps(hidden_states, Wq, bq, Wk, bk, Wv, bv, Wo):
    scale = 1.0 / math.sqrt(HD)
    bf = ml_dtypes.bfloat16
    masks = np.zeros((P, 4, 512), np.float32)
    kk = np.arange(P)[:, None]
    qq = np.arange(512)[None, :]
    for r in range(4):
        masks[:, r, :] = (qq >= kk + P * r).astype(np.float32)
    masks = masks.astype(bf)

    shard_maps = []
    for r in range(4):
        ds = slice(DSH * r, DSH * (r + 1))
        shard_maps.append(
            {
                "wqt": np.ascontiguousarray((Wq[ds, :] * scale).T).astype(bf),
                "wkt": np.ascontiguousarray(Wk[ds, :].T).astype(bf),
                "wvt": np.ascontiguousarray(Wv[ds, :].T).astype(bf),
                "wot": np.ascontiguousarray(Wo[:, ds].T).astype(bf),
                "bq2": np.ascontiguousarray(
                    (bq[ds] * scale).reshape(HPC, HD).T
                ).astype(np.float32),
                "bk2": np.ascontiguousarray(bk[ds].reshape(HPC, HD).T).astype(
                    np.float32
                ),
                "bvb": np.tile(bv[ds][None, :], (P, 1)).astype(np.float32),
                "msk": masks,
            }
        )

    hts = [
        np.ascontiguousarray(hidden_states[b].T).astype(bf) for b in range(B)
    ]

    in_maps = []
    for c in range(NCORES):
        b, r = divmod(c, 4)
        in_maps.append({"ht": hts[b], **shard_maps[r]})
    return in_maps


def _assemble_output(partials, bo):
    out = np.zeros((B, S, H), np.float32)
    for c in range(NCORES):
        out[c // 4] += partials[c].astype(np.float32)
    out += bo[None, None, :]
    return out


def kernel(hidden_states, attention_mask, Wq, bq, Wk, bk, Wv, bv, Wo, bo):
    hidden_states = np.asarray(hidden_states, dtype=np.float32)
    attention_mask = np.asarray(attention_mask, dtype=np.float32)
    Wq, bq = np.asarray(Wq, np.float32), np.asarray(bq, np.float32)
    Wk, bk = np.asarray(Wk, np.float32), np.asarray(bk, np.float32)
    Wv, bv = np.asarray(Wv, np.float32), np.asarray(bv, np.float32)
    Wo, bo = np.asarray(Wo, np.float32), np.asarray(bo, np.float32)

    if not _is_causal_mask(attention_mask):
        # The device kernel exploits the causal structure; any other mask
        # falls back to an exact host computation.
        return _reference_numpy(
            hidden_states, attention_mask, Wq, bq, Wk, bk, Wv, bv, Wo, bo
        )

    in_maps = _prepare_in_maps(hidden_states, Wq, bq, Wk, bk, Wv, bv, Wo)
    nc = _get_nc()
    for _attempt in range(2):
        res = run_bass_kernel_spmd(nc, in_maps, core_ids=list(range(NCORES)))
        out = _assemble_output([res.results[c]["o"] for c in range(NCORES)], bo)
        # a cold device has been observed to return garbage on the very
        # first execution; one retry clears it
        if np.isfinite(out).all():
            return out
    return out



# revision 23
# speedup vs baseline: 1.1252x; 1.1252x over previous
"""Multi-head causal self-attention (B=2, S=2048, H=2048, NH=16) on 8 TRN2
NeuronCores.

Sharding: data-parallel over batch (2 groups of 4 cores) x tensor-parallel
over heads (4 heads per core; q/k/v projections column-split, output
projection row-split). Each core computes a partial [S, H] output-projection
product; the host sums the 4 partials per batch and adds the output bias.

Per-core device kernel (all matmul inputs bf16, fp32 PSUM accumulation),
organized as one software-pipelined stream so the 2.4 GHz PE never waits
on the 1.2 GHz ACT engine or on DVE drains:
  - attention chunks (head, 512-wide q-chunk) in [k, q] orientation, one
    k-tile (128 keys) per single-bank PSUM tile from a 5-deep ring; the
    four diagonal k-tiles trim scores/exp/pv to the causal q-range
    (q_loc >= 128*t_loc), so only a shared [128,128] triangle mask ever
    multiplies; scores of tile k are emitted before the pv matmul of
    tile k-2
  - softmax denominator: bf16 pair-sums accumulated into an f32r esum on
    DVE (range-split on the diagonal pairs), then one f32r matmul per
    chunk whose all-ones [128,128] stationary operand reduces over
    partitions AND broadcasts the result; den matmul and the
    reciprocal+multiply normalize are deferred into the NEXT chunk's
    stream
  - j=0 phase-1 runs contraction-tile-major (4 PSUM accumulators per
    pass) so the cold-start PE idle is bounded by the DMA lead of one
    tile; later Q/K/V projection chains (1/sqrt(hd) folded into Wq on
    host) for round j+1 and output-projection row-tiles for round j-1
    ride as "filler quanta" popped between attention k-steps
  - the final den/normalize latency is hidden behind head-0..2 partial
    products of the first tail outproj quanta
  - PSUM->SBUF output drains alternate between ACT and DVE

_build_nc(reps=K) repeats the identical computation K times in one NEFF;
test.py measures the per-execution slope between two reps builds, which
cancels the multi-ms axon/PJRT dispatch overhead that would otherwise
swamp the ~285 us device time.
"""

import math
import sys

if "/opt/trn_rl_repo" not in sys.path:
    sys.path.insert(0, "/opt/trn_rl_repo")

import numpy as np
import ml_dtypes

import concourse.bass as bass
import concourse.mybir as mybir
import concourse.tile as tile
from concourse.bass_utils import run_bass_kernel_spmd

B, S, H, NH = 2, 2048, 2048, 16
HD = H // NH            # 128
NCORES = 8
HPC = NH // 4           # 4 heads per core
DSH = HPC * HD          # 512 per-core head-dim shard
P = 128                 # partitions
NT = S // P             # 16 s/k tiles of 128
NJ = S // 512           # 4 q/s chunks of 512
BF16 = mybir.dt.bfloat16
F32 = mybir.dt.float32

_NEG_BIG = -1.0e8  # masked entries in the reference mask are <= -1e9


def _split_excess_waits(nc, max_waits: int = 1) -> int:
    """This container's walrus rejects >1 sync wait per instruction
    ("Too many sync wait commands" in setupSyncWait). Hoist excess waits
    onto preceding same-engine NoOps; waits still execute in engine order
    before the original instruction, so sync semantics are unchanged."""
    n_split = 0
    for f in nc.m.functions:
        for bb in f.blocks:
            insts = bb.instructions
            out = []
            changed = False
            for inst in insts:
                si = inst.sync_info
                if si is not None and len(si.on_wait) > max_waits:
                    waits = list(si.on_wait)
                    excess, keep = waits[:-max_waits], waits[-max_waits:]
                    for i in range(0, len(excess), max_waits):
                        chunk = excess[i : i + max_waits]
                        nop = mybir.InstNoOp(
                            name=f"{inst.name}-waitsplit-{i}", ins=[], outs=[]
                        )
                        nop.engine = inst.engine
                        nop.sync_info = mybir.SyncInfo(on_wait=chunk, on_update=[])
                        nc.register_instruction(nop)
                        out.append(nop)
                    inst.sync_info = mybir.SyncInfo(
                        on_wait=keep, on_update=list(si.on_update)
                    )
                    changed = True
                    n_split += 1
                out.append(inst)
            if changed:
                bb.instructions = out
    return n_split


def _build_nc(reps: int = 1):
    """Build the kernel IR. reps>1 repeats the whole computation (including
    all DMA) back-to-back inside one NEFF — used by the timing harness to
    amortize the per-dispatch overhead of the axon/PJRT path; every rep
    performs identical work to the reps=1 program kernel() executes."""
    nc = bass.Bass()
    ht = nc.dram_tensor("ht", (H, S), BF16, kind="ExternalInput")
    wqt = nc.dram_tensor("wqt", (H, DSH), BF16, kind="ExternalInput")
    wkt = nc.dram_tensor("wkt", (H, DSH), BF16, kind="ExternalInput")
    wvt = nc.dram_tensor("wvt", (H, DSH), BF16, kind="ExternalInput")
    wot = nc.dram_tensor("wot", (DSH, H), BF16, kind="ExternalInput")
    bq2 = nc.dram_tensor("bq2", (P, HPC), F32, kind="ExternalInput")
    bk2 = nc.dram_tensor("bk2", (P, HPC), F32, kind="ExternalInput")
    bvb = nc.dram_tensor("bvb", (P, DSH), F32, kind="ExternalInput")
    msk = nc.dram_tensor("msk", (P, 4, 512), BF16, kind="ExternalInput")
    # partial products are summed on the host; bf16 partials halve the
    # output traffic and cost <0.1% relative error on the final sum
    o = nc.dram_tensor("o", (S, H), BF16, kind="ExternalOutput")

    with tile.TileContext(nc) as tc:
        with (
            tc.tile_pool(name="wpool", bufs=1) as wpool,
            tc.tile_pool(name="cpool", bufs=1) as cpool,
            tc.tile_pool(name="hpool", bufs=2) as hpool,
            tc.tile_pool(name="qkpool", bufs=1) as qkpool,
            tc.tile_pool(name="epool", bufs=6) as epool,
            tc.tile_pool(name="rpool", bufs=2) as rpool,
            tc.tile_pool(name="opool", bufs=4) as opool,
            # one shared pool of single-bank score/acc tiles + ot
            # accumulators + the den broadcast: 5 + 2 + 1 = 8 PSUM banks
            tc.tile_pool(name="ps_mm", bufs=5, space="PSUM") as ps_mm,
            tc.tile_pool(name="ps_out", bufs=2, space="PSUM") as ps_out,
            tc.tile_pool(name="ps_den", bufs=1, space="PSUM") as ps_den,
        ):
            for _rep in range(reps):
                _build_body(
                    nc, wpool, cpool, hpool, qkpool, epool, rpool, opool,
                    ps_mm, ps_out, ps_den,
                    ht, wqt, wkt, wvt, wot, bq2, bk2, bvb, msk, o,
                    warm=(_rep == 0),
                )

    _split_excess_waits(nc)
    return nc


def _build_body(
    nc, wpool, cpool, hpool, qkpool, epool, rpool, opool,
    ps_mm, ps_out, ps_den,
    ht, wqt, wkt, wvt, wot, bq2, bk2, bvb, msk, o,
    warm=False,
):
    if True:
        if True:
            if warm:
                # HAM pre-warm (first rep only): ~4us of dummy matmuls fill
                # the initial DMA-lead idle so the PE clock gate is already
                # at 8/8 (2.4 GHz) when the first real tile arrives. During
                # the DMA-paced startup a COLD tile-step (1.7us at 1.2 GHz)
                # is slower than the 1.16us DMA arrival rate, so the cold
                # penalty would otherwise NOT hide behind the transfers.
                # The reps-slope metric subtracts rep 0 entirely.
                wz = cpool.tile([P, P], BF16, tag="warmz")
                nc.vector.memset(wz[:], 0.0)
                wps = ps_den.tile([P, 512], F32, tag="den", name="warmps")
                for _i in range(40):
                    nc.tensor.matmul(
                        wps[:, 0:P], wz[:], wz[:],
                        start=True, stop=True, skip_group_check=True,
                    )

            # ---- constants / weights into SBUF ----
            # Load order matters: the first Q matmuls need wq + the first
            # hidden chunk; split the big loads in 4 so they spread across
            # DMA queues and compute starts as early as possible. wo is not
            # needed until phase 3 and is loaded right before it.
            wq_sb = wpool.tile([P, NT, DSH], BF16, tag="wq")
            wk_sb = wpool.tile([P, NT, DSH], BF16, tag="wk")
            wv_sb = wpool.tile([P, NT, DSH], BF16, tag="wv")
            wqt_r = wqt.rearrange("(t p) m -> p t m", p=P)
            wkt_r = wkt.rearrange("(t p) m -> p t m", p=P)
            wvt_r = wvt.rearrange("(t p) m -> p t m", p=P)
            # finest split for the first-needed tiles, alternating the two
            # tensors the first accumulation reads: the j=0 Q pass can begin
            # as soon as hidden tile 0 + wq tile 0 arrive
            h0_sb = hpool.tile([P, NT, 512], BF16, tag="h")
            ht_r0 = ht[:, 0:512].rearrange("(t p) s -> p t s", p=P)
            for t in range(NT):
                nc.sync.dma_start(h0_sb[:, t, :], ht_r0[:, t, :])
                nc.sync.dma_start(wq_sb[:, t, :], wqt_r[:, t, :])
            bq_sb = cpool.tile([P, HPC], F32, tag="bq")
            nc.sync.dma_start(bq_sb[:], bq2[:, :])
            bk_sb = cpool.tile([P, HPC], F32, tag="bk")
            nc.sync.dma_start(bk_sb[:], bk2[:, :])
            bv_sb = cpool.tile([P, DSH], F32, tag="bv")
            nc.sync.dma_start(bv_sb[:], bvb[:, :])
            mask_sb = cpool.tile([P, 4, 512], BF16, tag="msk")
            nc.sync.dma_start(mask_sb[:], msk[:, :, :])
            # all-ones [128,128] f32r matrix: the den matmul's stationary
            # operand, so its partition-reduction lands already broadcast
            # across all 128 output partitions (no separate rb matmul)
            ones_f32 = cpool.tile([P, P], F32, tag="ones32")
            nc.vector.memset(ones_f32[:], 1.0)
            onesmat_sb = cpool.tile([P, P], mybir.dt.float32r, tag="ones")
            nc.vector.tensor_copy(onesmat_sb[:], ones_f32[:])

            qt_sb = qkpool.tile([P, HPC, S], BF16, tag="qt")   # per-head Q^T [d, s]
            kt_sb = qkpool.tile([P, HPC, S], BF16, tag="kt")   # per-head K^T [d, s]
            v_sb = qkpool.tile([P, NT, DSH], BF16, tag="v")   # V [s-tile, d]
            ao_sb = qkpool.tile([P, HPC, S], BF16, tag="ao")   # attn-out^T [d, q] per head

            wo_sb = wpool.tile([P, HPC, H], BF16, tag="wo")
            wot_r = wot.rearrange("(t p) h -> p t h", p=P)

            h_tiles = {}

            def _phase1_dma(j):
                # kick off the hidden-chunk (and, for j=0, weight) loads
                sj = slice(512 * j, 512 * (j + 1))
                if j == 0:
                    h_tiles[0] = h0_sb
                    # K/V weights arrive while the j=0 Q pass computes; wo
                    # is issued in _phase1_dma(1) AFTER the h1 chunk, since
                    # h1 feeds round-0 fillers (~37us) while wo is first
                    # read by outproj(0) during round 1 (~70us)
                    for q4 in range(4):
                        t4 = slice(4 * q4, 4 * (q4 + 1))
                        nc.sync.dma_start(wk_sb[:, t4, :], wkt_r[:, t4, :])
                    for q4 in range(4):
                        t4 = slice(4 * q4, 4 * (q4 + 1))
                        nc.sync.dma_start(wv_sb[:, t4, :], wvt_r[:, t4, :])
                else:
                    h_sb = hpool.tile([P, NT, 512], BF16, tag="h")
                    ht_r = ht[:, sj].rearrange("(t p) s -> p t s", p=P)
                    for q4 in range(4):
                        t4 = slice(4 * q4, 4 * (q4 + 1))
                        nc.sync.dma_start(h_sb[:, t4, :], ht_r[:, t4, :])
                    h_tiles[j] = h_sb
                    if j == 1:
                        for q4 in range(4):
                            nc.sync.dma_start(wo_sb[:, q4, :], wot_r[:, q4, :])

            # ---- filler quanta ----
            # Phase-1 projection chains and output-projection groups are
            # wrapped in small closures ("quanta", ~0.9-3.4us of PE work
            # each) and popped from a FIFO after every attention pair-step.
            # These PE-only stretches between score/PV steps let the 1.2 GHz
            # ACT engine (which can never outrun the 2.4 GHz PE within a
            # contiguous run of attention steps) catch up on its exp
            # backlog, and widen every PSUM-buffer / DVE-drain recycle
            # window. Accumulators are single-bank [P,512] tiles from the
            # shared ps_mm ring.
            filler_q = []

            def _fill_half():
                return ps_mm.tile([P, 512], F32, tag="mm", name="fillacc")

            def _pop_filler(n=1):
                for _ in range(n):
                    if not filler_q:
                        return
                    filler_q.pop(0)()

            def _drain_fillers():
                while filler_q:
                    filler_q.pop(0)()

            def _q_quantum(j, hd, h_sb):
                def f():
                    sj = slice(512 * j, 512 * (j + 1))
                    md = slice(HD * hd, HD * (hd + 1))
                    acc = _fill_half()
                    for t in range(NT):
                        nc.tensor.matmul(
                            acc, wq_sb[:, t, md], h_sb[:, t, :],
                            start=(t == 0), stop=(t == NT - 1),
                            skip_group_check=True,
                        )
                    nc.vector.tensor_scalar_add(
                        qt_sb[:, hd, sj], acc, bq_sb[:, hd : hd + 1]
                    )
                return f

            def _k_quantum(j, hd, h_sb):
                def f():
                    sj = slice(512 * j, 512 * (j + 1))
                    md = slice(HD * hd, HD * (hd + 1))
                    acc = _fill_half()
                    for t in range(NT):
                        nc.tensor.matmul(
                            acc, wk_sb[:, t, md], h_sb[:, t, :],
                            start=(t == 0), stop=(t == NT - 1),
                            skip_group_check=True,
                        )
                    nc.vector.tensor_scalar_add(
                        kt_sb[:, hd, sj], acc, bk_sb[:, hd : hd + 1]
                    )
                return f

            def _v_quantum(j, st, h_sb):
                def f():
                    ms = slice(P * st, P * (st + 1))
                    acc = _fill_half()
                    for t in range(NT):
                        nc.tensor.matmul(
                            acc, h_sb[:, t, ms], wv_sb[:, t, :],
                            start=(t == 0), stop=(t == NT - 1),
                            skip_group_check=True,
                        )
                    nc.vector.tensor_add(v_sb[:, 4 * j + st, :], acc, bv_sb[:])
                return f

            _oc_flip = [0]
            _tail_mode = [False]

            def _oc_drain(acc, rs, hc):
                oc = opool.tile([P, 512], BF16, tag="oc")
                # PSUM->SBUF drains alternate ACT/DVE so neither queue
                # grows enough to delay release-critical ops (exp on
                # ACT; esum / normalize on DVE). In the post-attention
                # tail both exp and normalize are done, and DVE still
                # holds hoisted waits — route every drain to the idle ACT
                # so accumulator banks recycle as fast as possible.
                _oc_flip[0] ^= 1
                if _tail_mode[0] or _oc_flip[0]:
                    nc.scalar.activation(
                        oc[:], acc, mybir.ActivationFunctionType.Copy
                    )
                else:
                    nc.vector.tensor_copy(oc[:], acc)
                nc.sync.dma_start(o[rs, hc], oc[:])

            def _o_quantum(si, c):
                def f():
                    rs = slice(P * si, P * (si + 1))
                    hc = slice(512 * c, 512 * (c + 1))
                    acc = _fill_half()
                    for dt in range(HPC):
                        nc.tensor.matmul(
                            acc, ao_sb[:, dt, rs], wo_sb[:, dt, hc],
                            start=(dt == 0), stop=(dt == HPC - 1),
                            skip_group_check=True,
                        )
                    _oc_drain(acc, rs, hc)
                return f

            def _queue_phase1(j):
                # kind-major order: at j=0 this matches the DMA arrival
                # order (wq -> wk -> wv), so the PE never waits on a weight
                # tensor whose load was issued after one it already consumed
                h_sb = h_tiles.pop(j)
                for hd in range(HPC):
                    filler_q.append(_q_quantum(j, hd, h_sb))
                for hd in range(HPC):
                    filler_q.append(_k_quantum(j, hd, h_sb))
                for hd in range(HPC):
                    filler_q.append(_v_quantum(j, hd, h_sb))

            def _queue_outproj(j):
                for si in range(4 * j, 4 * (j + 1)):
                    for c in range(NJ):
                        filler_q.append(_o_quantum(si, c))

            # pending[0]: previous chunk awaiting its den matmul + normalize;
            # both are deferred into the NEXT chunk's PE stream so the PE
            # never stalls on the ACT-exp / DVE-esum latency behind them
            pending = [None]

            def _flush_den():
                # den matmul for the previous chunk: ONE f32r matmul whose
                # all-ones [128,128] stationary operand both reduces the
                # DVE-accumulated esum over partitions AND broadcasts the
                # result to every output partition (kmax x cheaper on PE
                # than [1,512] ones-matmuls per k-tile, and no rb matmul)
                ot_ps, esum_r, n_hd, n_sj, den_ref = pending[0]
                den_ps = ps_den.tile([P, 512], F32, tag="den")
                nc.tensor.matmul(
                    den_ps[:], onesmat_sb[:], esum_r[:], start=True, stop=True
                )
                den_ref.append(den_ps)

            def _normalize():
                # divide the accumulated outT by the softmax denominator:
                # reciprocal of the broadcast den on DVE, then a multiply
                # into the bf16 attn-out tile (reciprocal_approx_fast would
                # shave ~500ns but its custom DVE ISA op fails this
                # container's walrus codegen: "ISA wrong length")
                ot_ps, _, n_hd, n_sj, den_ref = pending[0]
                rb = rpool.tile([P, 512], F32, tag="rb")
                nc.vector.reciprocal(rb[:], den_ref[0][:])
                nc.vector.tensor_mul(ao_sb[:, n_hd, n_sj], ot_ps[:], rb[:])
                pending[0] = None

            def _attn_chunk(hd, j):
                # causal attention for (head hd, q-chunk j), [k, q]
                # orientation, one k-tile (128 keys) per PSUM bank. The four
                # diagonal k-tiles (the last four) restrict the moving q
                # range to q_loc >= 128*t_loc: scores, exp and pv all skip
                # the strictly-upper-triangle 512-blocks, and only the
                # 128-wide diagonal block needs a (shared) triangle mask.
                sj0 = 512 * j
                kmax = 4 * (j + 1)
                md = slice(HD * hd, HD * (hd + 1))
                trims = [max(0, 128 * (k - (kmax - 4))) for k in range(kmax)]
                flush_k = min(5, kmax - 1)
                ot_ps = ps_out.tile([P, 512], F32, tag="ot")
                # accumulated directly in f32r (the den matmul's moving
                # operand dtype): every DVE writer rounds to f32r, so no
                # converting copy is needed before the den matmul
                esum = rpool.tile([P, 512], mybir.dt.float32r, tag="esum")
                # PE stream is in-order: emit the scores of tile k BEFORE the
                # pv matmul of tile k-2 so the PE streams scores while ACT
                # computes exp, and slot the previous chunk's den matmul /
                # normalize into fixed early positions
                etiles = [None] * kmax
                for k in range(kmax + 2):
                    if k < kmax:
                        a = trims[k]
                        kd = slice(P * k, P * (k + 1))
                        st = ps_mm.tile([P, 512], F32, tag="mm")
                        nc.tensor.matmul(
                            st[:, a:], kt_sb[:, hd, kd],
                            qt_sb[:, hd, sj0 + a : sj0 + 512],
                            start=True, stop=True,
                            skip_group_check=True,
                        )
                        e = epool.tile([P, 512], BF16, tag="e")
                        nc.scalar.activation(
                            e[:, a:], st[:, a:],
                            mybir.ActivationFunctionType.Exp,
                        )
                        if k >= kmax - 4:
                            # triangle mask on the 128-wide diagonal block
                            nc.vector.tensor_mul(
                                e[:, a : a + 128], e[:, a : a + 128],
                                mask_sb[:, 0, 0:128],
                            )
                        etiles[k] = e
                        if k % 2 == 1:
                            # denominator: bf16 pair-sum over the region both
                            # tiles cover, direct add where only the earlier
                            # tile is valid, then f32 accumulate
                            ep, ac = etiles[k - 1], a
                            pb = rpool.tile([P, 512], BF16, tag="pb")
                            if ac == 0:
                                nc.vector.tensor_add(pb[:], ep[:], e[:])
                                if k == 1:
                                    nc.vector.tensor_copy(esum[:], pb[:])
                                else:
                                    nc.vector.tensor_add(esum[:], esum[:], pb[:])
                            else:
                                ap = ac - 128
                                nc.vector.tensor_add(
                                    pb[:, ac:], ep[:, ac:], e[:, ac:]
                                )
                                if k == 1:
                                    nc.vector.tensor_copy(
                                        esum[:, ap:ac], ep[:, ap:ac]
                                    )
                                    nc.vector.tensor_copy(
                                        esum[:, ac:], pb[:, ac:]
                                    )
                                else:
                                    nc.vector.tensor_add(
                                        esum[:, ap:ac], esum[:, ap:ac],
                                        ep[:, ap:ac],
                                    )
                                    nc.vector.tensor_add(
                                        esum[:, ac:], esum[:, ac:], pb[:, ac:]
                                    )
                        if k == flush_k and pending[0] is not None:
                            _flush_den()
                    if k >= 2:
                        kk = k - 2
                        a = trims[kk]
                        nc.tensor.matmul(
                            ot_ps[:, a:], v_sb[:, kk, md], etiles[kk][:, a:],
                            start=(kk == 0), stop=(kk == kmax - 1),
                            skip_group_check=True,
                        )
                        if kk == flush_k and pending[0] is not None:
                            _normalize()
                            if on_norm is not None:
                                on_norm()
                    if k % 2 == 1:
                        _pop_filler()
                pending[0] = (ot_ps, esum, hd, slice(sj0, sj0 + 512), [])

            def _phase1_j0_tilemajor():
                # j=0 phase-1 in contraction-tile-major order: each arriving
                # (h0[t], w[t]) DMA pair unlocks HPC matmuls immediately, so
                # the cold-start PE idle is bounded by the DMA lead of a
                # single tile instead of a whole tensor. Four accumulators
                # (one per head / s-tile) live in the 5-deep ps_mm ring.
                h_sb = h_tiles.pop(0)
                for kind in range(3):
                    w_sb = (wq_sb, wk_sb, wv_sb)[kind]
                    accs = [
                        ps_mm.tile([P, 512], F32, tag="mm", name="p1acc")
                        for _ in range(HPC)
                    ]
                    for t in range(NT):
                        for i in range(HPC):
                            if kind == 2:
                                # V: stationary = hidden s-tile, moving = wv
                                nc.tensor.matmul(
                                    accs[i], h_sb[:, t, P * i : P * (i + 1)],
                                    w_sb[:, t, :],
                                    start=(t == 0), stop=(t == NT - 1),
                                    skip_group_check=True,
                                )
                            else:
                                nc.tensor.matmul(
                                    accs[i], w_sb[:, t, HD * i : HD * (i + 1)],
                                    h_sb[:, t, :],
                                    start=(t == 0), stop=(t == NT - 1),
                                    skip_group_check=True,
                                )
                    for i in range(HPC):
                        if kind == 0:
                            nc.vector.tensor_scalar_add(
                                qt_sb[:, i, 0:512], accs[i], bq_sb[:, i : i + 1]
                            )
                        elif kind == 1:
                            nc.vector.tensor_scalar_add(
                                kt_sb[:, i, 0:512], accs[i], bk_sb[:, i : i + 1]
                            )
                        else:
                            nc.vector.tensor_add(v_sb[:, i, :], accs[i], bv_sb[:])

            # Software pipeline: phase-1 of round j+1 and the output
            # projection of round j-1 ride as filler quanta inside round j's
            # attention chunks (K/V tiles of round j only reach s <= 512(j+1)
            # by causality, and outproj j-1 unblocks once chunk(0, j) flushes
            # the last pending normalize of round j-1).
            _phase1_dma(0)
            if NJ > 1:
                _phase1_dma(1)
            _phase1_j0_tilemajor()
            for j in range(NJ):
                # h for round j+2 streams in while round j computes; the
                # phase-1 quanta of round j+1 queued here read the h chunk
                # that already landed during round j-1
                if j + 2 < NJ:
                    _phase1_dma(j + 2)
                if j + 1 < NJ:
                    _queue_phase1(j + 1)
                for hd in range(HPC):
                    on_norm = None
                    if hd == 0 and j > 0:
                        jj = j - 1
                        on_norm = lambda jj=jj: _queue_outproj(jj)
                    _attn_chunk(hd, j)
                # round j+1's chunks need all of phase-1(j+1): drain whatever
                # the pair-step slots didn't absorb (outproj quanta may spill)
                if j + 1 < NJ:
                    _drain_fillers()
            # Tail: the final den matmul waits ~1.5us on the DVE esum chain
            # of chunk (3, NJ-1). Emit the head-0..2 partial products of the
            # first few outproj quanta ahead of it in PE program order (they
            # only read already-normalized heads), then finish them (head-3
            # matmul + drain) after the last normalize.
            def _o_quantum_partial(si, c):
                rs = slice(P * si, P * (si + 1))
                hc = slice(512 * c, 512 * (c + 1))
                acc = _fill_half()
                for dt in range(HPC - 1):
                    nc.tensor.matmul(
                        acc, ao_sb[:, dt, rs], wo_sb[:, dt, hc],
                        start=(dt == 0), stop=False,
                        skip_group_check=True,
                    )

                def finish():
                    nc.tensor.matmul(
                        acc, ao_sb[:, HPC - 1, rs], wo_sb[:, HPC - 1, hc],
                        start=False, stop=True,
                        skip_group_check=True,
                    )
                    _oc_drain(acc, rs, hc)

                return finish

            # 5 partials = the ps_mm ring size; a 6th open accumulator would
            # deadlock against its own finisher's drain in the PE FIFO.
            _tail_mode[0] = True
            sis = 4 * (NJ - 1)
            finishers = [_o_quantum_partial(sis, c) for c in range(3)]
            _flush_den()
            _normalize()
            finishers.append(_o_quantum_partial(sis, 3))
            finishers.append(_o_quantum_partial(sis + 1, 0))
            for f in finishers:
                f()
            for c in range(1, NJ):
                filler_q.append(_o_quantum(sis + 1, c))
            for si in range(sis + 2, sis + 4):
                for c in range(NJ):
                    filler_q.append(_o_quantum(si, c))
            _drain_fillers()

    _split_excess_waits(nc)
    return nc


_NC_CACHE = None


def _get_nc():
    global _NC_CACHE
    if _NC_CACHE is None:
        _NC_CACHE = _build_nc()
    return _NC_CACHE


def _is_causal_mask(mask: np.ndarray) -> bool:
    if mask.shape != (1, 1, S, S):
        return False
    m = mask[0, 0]
    tri = np.tril(np.ones((S, S), dtype=bool))
    return bool(np.all(m[tri] == 0.0) and np.all(m[~tri] <= _NEG_BIG))


def _reference_numpy(hidden_states, attention_mask, Wq, bq, Wk, bk, Wv, bv, Wo, bo):
    hs = hidden_states.astype(np.float64)
    out = np.empty((B, S, H), np.float64)
    for b in range(B):
        q = hs[b] @ Wq.T.astype(np.float64) + bq
        k = hs[b] @ Wk.T.astype(np.float64) + bk
        v = hs[b] @ Wv.T.astype(np.float64) + bv
        q = q.reshape(S, NH, HD).transpose(1, 0, 2)
        k = k.reshape(S, NH, HD).transpose(1, 0, 2)
        v = v.reshape(S, NH, HD).transpose(1, 0, 2)
        attn = np.einsum("nqd,nkd->nqk", q, k) / math.sqrt(HD)
        attn = attn + attention_mask[0].astype(np.float64)
        attn = attn - attn.max(axis=-1, keepdims=True)
        attn = np.exp(attn)
        attn = attn / attn.sum(axis=-1, keepdims=True)
        o = np.einsum("nqk,nkd->nqd", attn, v)
        o = o.transpose(1, 0, 2).reshape(S, H)
        out[b] = o @ Wo.T.astype(np.float64) + bo
    return out.astype(np.float32)


def _prepare_in_maps(hidden_states, Wq, bq, Wk, bk, Wv, bv, Wo):
    scale = 1.0 / math.sqrt(HD)
    bf = ml_dtypes.bfloat16
    masks = np.zeros((P, 4, 512), np.float32)
    kk = np.arange(P)[:, None]
    qq = np.arange(512)[None, :]
    for r in range(4):
        masks[:, r, :] = (qq >= kk + P * r).astype(np.float32)
    masks = masks.astype(bf)

    shard_maps = []
    for r in range(4):
        ds = slice(DSH * r, DSH * (r + 1))
        shard_maps.append(
            {
                "wqt": np.ascontiguousarray((Wq[ds, :] * scale).T).astype(bf),
                "wkt": np.ascontiguousarray(Wk[ds, :].T).astype(bf),
                "wvt": np.ascontiguousarray(Wv[ds, :].T).astype(bf),
                "wot": np.ascontiguousarray(Wo[:, ds].T).astype(bf),
                "bq2": np.ascontiguousarray(
                    (bq[ds] * scale).reshape(HPC, HD).T
                ).astype(np.float32),
                "bk2": np.ascontiguousarray(bk[ds].reshape(HPC, HD).T).astype(
                    np.float32
                ),
                "bvb": np.tile(bv[ds][None, :], (P, 1)).astype(np.float32),
                "msk": masks,
            }
        )

    hts = [
        np.ascontiguousarray(hidden_states[b].T).astype(bf) for b in range(B)
    ]

    in_maps = []
    for c in range(NCORES):
        b, r = divmod(c, 4)
        in_maps.append({"ht": hts[b], **shard_maps[r]})
    return in_maps


def _assemble_output(partials, bo):
    out = np.zeros((B, S, H), np.float32)
    for c in range(NCORES):
        out[c // 4] += partials[c].astype(np.float32)
    out += bo[None, None, :]
    return out


def kernel(hidden_states, attention_mask, Wq, bq, Wk, bk, Wv, bv, Wo, bo):
    hidden_states = np.asarray(hidden_states, dtype=np.float32)
    attention_mask = np.asarray(attention_mask, dtype=np.float32)
    Wq, bq = np.asarray(Wq, np.float32), np.asarray(bq, np.float32)
    Wk, bk = np.asarray(Wk, np.float32), np.asarray(bk, np.float32)
    Wv, bv = np.asarray(Wv, np.float32), np.asarray(bv, np.float32)
    Wo, bo = np.asarray(Wo, np.float32), np.asarray(bo, np.float32)

    if not _is_causal_mask(attention_mask):
        # The device kernel exploits the causal structure; any other mask
        # falls back to an exact host computation.
        return _reference_numpy(
            hidden_states, attention_mask, Wq, bq, Wk, bk, Wv, bv, Wo, bo
        )

    in_maps = _prepare_in_maps(hidden_states, Wq, bq, Wk, bk, Wv, bv, Wo)
    nc = _get_nc()
    for _attempt in range(2):
        res = run_bass_kernel_spmd(nc, in_maps, core_ids=list(range(NCORES)))
        out = _assemble_output([res.results[c]["o"] for c in range(NCORES)], bo)
        # a cold device has been observed to return garbage on the very
        # first execution; one retry clears it
        if np.isfinite(out).all():
            return out
    return out

